# revision 1
# baseline (speedup 1.0000x reference)
"""Trainium2 Bass kernel for nn_Attention (B=16,N=4096,C=1024,H=16,HD=64,Q=64).

Data-parallel over B across 8 NeuronCores (2 batches/core). Per batch the
attention is reassociated so no k/v tensors are materialized and no on-chip
transposes are needed:

  q^T = Wq @ x_q^T                      [(h,d)=1024, 64]
  G_h^T = Wk_h^T @ q_h                  G^T: [c=1024, (h,q)=1024]
  S^T   = x @ G^T   (per t-tile)        [t, (h,q)]   (contract c)
  p^T   = exp(S^T / 8)                  (softmax w/o max-sub: scores ~ +-5)
  u^T   = x^T(nat) @ p^T  (accum t)     [c, (h,q)]   (contract t)
  den   = ones @ pacc     (pacc: DVE/GpSimd p-sum over t)
  o_h^T = (Wv_h^T)^T @ u_h^T, scaled by 1/den at PSUM eviction
  y     = o^T.T @ Wproj^T + b           [64, 1024]   (contract (h,d))

Host feeds per core: x natural + x transposed, Wq^T, Wk, Wv^T, Wproj^T, b —
x/weights in bf16; all matmuls bf16 with fp32 PSUM accumulation, N=512.
Weights load once per core; both batches' prologues run up front so the
batch boundary keeps the PE fed.
"""
import os
import numpy as np

SKIP = set(os.environ.get("ATT_SKIP", "").split(","))

B, N, C = 16, 4096, 1024
H, HD, QL = 16, 64, 64
BL = B // 8           # batches per core
CK = C // 128         # 8 c-tiles
TB = 512              # tokens per t-block
NBLK = N // TB        # 8 blocks
TPB = TB // 128       # 4 t-tiles per block
HQ = H * QL           # 1024
SCALE = HD ** -0.5

_CACHE = {}


def _build():
    import concourse.bass as bass
    import concourse.tile as tile
    from concourse import bacc, mybir

    f32 = mybir.dt.float32
    bf16 = mybir.dt.bfloat16
    EXP = mybir.ActivationFunctionType.Exp

    nc = bacc.Bacc("TRN2", target_bir_lowering=False, debug=False, num_devices=8)
    xn = nc.dram_tensor("xn", [BL, N, C], bf16, kind="ExternalInput").ap()
    xt = nc.dram_tensor("xt", [BL, C, N], bf16, kind="ExternalInput").ap()
    wq = nc.dram_tensor("wq", [C, C], bf16, kind="ExternalInput").ap()   # Wq^T
    wk = nc.dram_tensor("wk", [C, C], bf16, kind="ExternalInput").ap()   # Wk natural
    wv = nc.dram_tensor("wv", [C, C], bf16, kind="ExternalInput").ap()   # Wv^T
    wp = nc.dram_tensor("wp", [C, C], bf16, kind="ExternalInput").ap()   # Wproj^T
    bp = nc.dram_tensor("bp", [1, C], f32, kind="ExternalInput").ap()
    xq = nc.dram_tensor("xq", [BL, C, QL], bf16, kind="ExternalInput").ap()
    y = nc.dram_tensor("y", [BL, QL, C], f32, kind="ExternalOutput").ap()

    with tile.TileContext(nc) as tc:
        with (
            tc.tile_pool(name="wpool", bufs=2) as wpool,
            tc.tile_pool(name="xpool", bufs=2) as xpool,
            tc.tile_pool(name="gpool", bufs=1) as gpool,
            tc.tile_pool(name="upool", bufs=1) as upool,
            tc.tile_pool(name="small", bufs=1) as small,
            tc.tile_pool(name="ptp", bufs=3) as ptp,
            tc.tile_pool(name="psa", bufs=4, space="PSUM") as psa,
            tc.tile_pool(name="psu", bufs=4, space="PSUM") as psu,
        ):
            ones32 = small.tile([128, 8], f32, tag="ones32")
            nc.gpsimd.memset(ones32[:], 1.0)
            bps = small.tile([128, C], bf16, tag="bps")
            nc.gpsimd.dma_start(bps[0:1, :], bp[:, :])
            bpf = small.tile([128, C], bf16, tag="bpf")
            nc.gpsimd.partition_broadcast(bpf[:], bps[0:1, :])

            # ---------- weights: load once per core ----------
            wt = wpool.tile([128, 8 * 1024], bf16, tag="w", name="wt_q")
            for ck in range(CK):
                nc.sync.dma_start(wt[:, ck * 1024:(ck + 1) * 1024],
                                  wq[ck * 128:(ck + 1) * 128, :])
            wt2 = wpool.tile([128, 8 * 1024], bf16, tag="w", name="wt_k")
            for jc in range(CK):
                nc.sync.dma_start(wt2[:, jc * 1024:(jc + 1) * 1024],
                                  wk[jc * 128:(jc + 1) * 128, :])

            # ---------- both batches' prologues up front ----------
            gts = []
            for b in range(BL):
                xqt = small.tile([128, CK * QL], bf16, tag="xqt", bufs=2,
                                 name=f"xqt{b}")
                for ck in range(CK):
                    nc.sync.dma_start(xqt[:, ck * QL:(ck + 1) * QL],
                                      xq[b, ck * 128:(ck + 1) * 128, :])

                # q^T chunks land directly into the block-diagonal layout:
                # chunk jc rows = heads (2jc, 2jc+1); all G matmuls K=128 base 0
                qbd = small.tile([128, 8 * 128], bf16, tag="qbd", bufs=2,
                                 name=f"qbd{b}")
                nc.gpsimd.memset(qbd[:], 0.0)
                for jc in range(8):
                    ps = psa.tile([128, 512], f32, tag="psa")
                    for ck in range(CK):
                        nc.tensor.matmul(
                            ps[:, 0:QL],
                            wt[:, ck * 1024 + jc * 128: ck * 1024 + (jc + 1) * 128],
                            xqt[:, ck * QL:(ck + 1) * QL],
                            start=(ck == 0), stop=(ck == CK - 1))
                    nc.vector.tensor_copy(
                        qbd[0:64, jc * 128: jc * 128 + 64], ps[0:64, 0:QL])
                    nc.vector.tensor_copy(
                        qbd[64:128, jc * 128 + 64: (jc + 1) * 128], ps[64:128, 0:QL])

                # G^T [c,(h,q)] bf16: [128, CK*1024], c-tile ck at cols ck*1024
                gt = gpool.tile([128, CK * 1024], bf16, tag="gt", bufs=2,
                                name=f"gt{b}")
                for ck in range(CK):
                    for half in range(2):
                        ps = psa.tile([128, 512], f32, tag="psa")
                        for pp in range(4):
                            pair = half * 4 + pp
                            nc.tensor.matmul(
                                ps[:, pp * 128:(pp + 1) * 128],
                                wt2[:, pair * 1024 + ck * 128:
                                    pair * 1024 + (ck + 1) * 128],
                                qbd[:, pair * 128:(pair + 1) * 128],
                                start=True, stop=True)
                        nc.vector.tensor_copy(
                            gt[:, ck * 1024 + half * 512: ck * 1024 + (half + 1) * 512],
                            ps[:])
                gts.append(gt)

            # epilogue weights (reuse the two wpool slots; prefetch during t-loop)
            wt3 = wpool.tile([128, 8 * 1024], bf16, tag="w", name="wt_v")
            for ck in range(CK):
                nc.sync.dma_start(wt3[:, ck * 1024:(ck + 1) * 1024],
                                  wv[ck * 128:(ck + 1) * 128, :])
            wt4 = wpool.tile([128, 8 * 1024], bf16, tag="w", name="wt_p")
            for jc in range(CK):
                nc.sync.dma_start(wt4[:, jc * 1024:(jc + 1) * 1024],
                                  wp[jc * 128:(jc + 1) * 128, :])

            for b in range(BL):
                gt = gts[b]
                # ---------- t-loop ----------
                # u accumulator: [128, ck*1024 + qh*512 + hq%512] fp32
                ut = upool.tile([128, CK * 1024], f32, tag="ut", name=f"ut{b}")
                # unnormalized bf16 u written by the final eviction pass
                un = gpool.tile([128, CK * 1024], bf16, tag="un", name=f"un{b}")
                # p-sum accumulator for the softmax denominators
                pacc = small.tile([128, HQ], f32, tag="pacc", bufs=2,
                                  name=f"pacc{b}")
                nc.gpsimd.memset(pacc[:], 0.0)

                ptc_prev = None
                xnt_prev = None
                for blk in range(NBLK if "tloop" not in SKIP else 0):
                    xnt = xpool.tile([128, TPB * 1024], bf16, tag="xn")
                    for i in range(TPB):
                        nc.sync.dma_start(
                            xnt[:, i * 1024:(i + 1) * 1024],
                            xn[b, (blk * TPB + i) * 128:(blk * TPB + i + 1) * 128, :])
                    xtt = xpool.tile([128, CK * TB], bf16, tag="xt")
                    for ck in range(CK):
                        nc.sync.dma_start(
                            xtt[:, ck * TB:(ck + 1) * TB],
                            xt[b, ck * 128:(ck + 1) * 128, blk * TB:(blk + 1) * TB])

                    # S^T + exp into p cache; pacc accumulates p on GpSimd
                    ptc = ptp.tile([128, TPB * 1024], bf16, tag="ptc")
                    for i in range(TPB):
                        for qh in range(2):
                            st = psa.tile([128, 512], f32, tag="psa")
                            for ck in range(CK):
                                nc.tensor.matmul(
                                    st[:],
                                    xtt[:, ck * TB + i * 128: ck * TB + (i + 1) * 128],
                                    gt[:, ck * 1024 + qh * 512: ck * 1024 + (qh + 1) * 512],
                                    start=(ck == 0), stop=(ck == CK - 1))
                            pslice = ptc[:, i * 1024 + qh * 512: i * 1024 + (qh + 1) * 512]
                            nc.scalar.activation(pslice, st[:], EXP, scale=SCALE)
                            pa = pacc[:, qh * 512:(qh + 1) * 512]
                            nc.gpsimd.tensor_add(pa, pslice, pa)

                    # u^T accumulation: per (qh, c-quarter) round over TWO
                    # blocks, N=512 (halves the PSUM->SBUF eviction count)
                    if blk % 2 == 0:
                        ptc_prev, xnt_prev = ptc, xnt
                        continue
                    for qh in range(2):
                        for cq in range(2):
                            ups = [psu.tile([128, 512], f32, tag="ups",
                                            name=f"ups{b}_{blk}_{qh}_{cq}_{j}")
                                   for j in range(4)]
                            for half, (pp, xx) in enumerate([(ptc_prev, xnt_prev),
                                                             (ptc, xnt)]):
                                for i in range(TPB):
                                    for k4 in range(4):
                                        ck = cq * 4 + k4
                                        nc.tensor.matmul(
                                            ups[k4][:],
                                            xx[:, i * 1024 + ck * 128: i * 1024 + (ck + 1) * 128],
                                            pp[:, i * 1024 + qh * 512: i * 1024 + (qh + 1) * 512],
                                            start=(half == 0 and i == 0),
                                            stop=(half == 1 and i == TPB - 1))
                            for k4 in range(4):
                                ck = cq * 4 + k4
                                dst = ut[:, ck * 1024 + qh * 512: ck * 1024 + (qh + 1) * 512]
                                if blk == 1:
                                    nc.vector.tensor_copy(dst, ups[k4][:])
                                elif blk == NBLK - 1:
                                    nc.vector.tensor_add(
                                        un[:, ck * 1024 + qh * 512: ck * 1024 + (qh + 1) * 512],
                                        ups[k4][:], dst)
                                else:
                                    nc.vector.tensor_add(dst, ups[k4][:], dst)

                # ---------- epilogue ----------
                rd = small.tile([128, HQ], f32, tag="rd", name=f"rd{b}")
                for qh in range(2):
                    dnp = psa.tile([128, 512], f32, tag="psa", name=f"dnp{b}_{qh}")
                    nc.tensor.matmul(dnp[0:8, :], ones32[:],
                                     pacc[:, qh * 512:(qh + 1) * 512],
                                     start=True, stop=True)
                    nc.vector.reciprocal(rd[0:1, qh * 512:(qh + 1) * 512],
                                         dnp[0:1, :])
                rdf = small.tile([128, HQ], f32, tag="rdf", name=f"rdf{b}")
                nc.gpsimd.partition_broadcast(rdf[:], rd[0:1, :])
                # per-head-pair reciprocal layout for the oT scale:
                # rdo[p, jc*64+qq] = 1/d[(2jc + p//64)*64 + qq]
                rdo = small.tile([128, 8 * QL], f32, tag="rdo", name=f"rdo{b}")
                for jc in range(8):
                    nc.vector.tensor_copy(
                        rdo[0:64, jc * QL:(jc + 1) * QL],
                        rdf[0:64, (2 * jc) * QL:(2 * jc + 1) * QL])
                    nc.vector.tensor_copy(
                        rdo[64:128, jc * QL:(jc + 1) * QL],
                        rdf[64:128, (2 * jc + 1) * QL:(2 * jc + 2) * QL])

                oT = small.tile([128, 8 * QL], bf16, tag="oT", name=f"oT{b}")
                for jc in range(8):  # head pair (2jc, 2jc+1)
                    ps = psa.tile([128, 512], f32, tag="psa")
                    for sub in range(2):
                        h = jc * 2 + sub
                        ucol = (h // 8) * 512 + (h % 8) * 64
                        for ck in range(CK):
                            nc.tensor.matmul(
                                ps[sub * 64:(sub + 1) * 64, 0:QL],
                                wt3[:, ck * 1024 + h * 64: ck * 1024 + (h + 1) * 64],
                                un[:, ck * 1024 + ucol: ck * 1024 + ucol + 64],
                                start=(ck == 0), stop=(ck == CK - 1),
                                tile_position=(0, sub * 64))
                    nc.vector.tensor_mul(oT[:, jc * QL:(jc + 1) * QL],
                                         ps[:, 0:QL], rdo[:, jc * QL:(jc + 1) * QL])

                ys = small.tile([128, C], f32, tag="ys", name=f"ys{b}")
                for half in range(2):
                    ps = psa.tile([128, 512], f32, tag="psa")
                    for jc in range(8):
                        nc.tensor.matmul(
                            ps[0:QL, :],
                            oT[:, jc * QL:(jc + 1) * QL],
                            wt4[:, jc * 1024 + half * 512: jc * 1024 + (half + 1) * 512],
                            start=(jc == 0), stop=(jc == 7))
                    nc.vector.tensor_add(
                        ys[0:QL, half * 512:(half + 1) * 512], ps[0:QL, :],
                        bpf[0:QL, half * 512:(half + 1) * 512])
                nc.sync.dma_start(y[b, :, :], ys[0:QL, :])

    nc.compile()
    return nc


def get_nc():
    if "nc" not in _CACHE:
        _CACHE["nc"] = _build()
    return _CACHE["nc"]


def make_in_maps(x, Wq, Wk, Wv, Wproj, bproj):
    import ml_dtypes
    bf = ml_dtypes.bfloat16
    x = np.ascontiguousarray(x, dtype=np.float32)
    xt32 = np.ascontiguousarray(x.transpose(0, 2, 1))
    xqb = np.ascontiguousarray(xt32[:, :, 0:QL]).astype(bf)
    xtb = xt32.astype(bf)
    xnb = x.astype(bf)
    wqb = np.ascontiguousarray(np.asarray(Wq, dtype=np.float32).T).astype(bf)
    wkb = np.ascontiguousarray(np.asarray(Wk, dtype=np.float32)).astype(bf)
    wvb = np.ascontiguousarray(np.asarray(Wv, dtype=np.float32).T).astype(bf)
    wpb = np.ascontiguousarray(np.asarray(Wproj, dtype=np.float32).T).astype(bf)
    bpf = np.ascontiguousarray(np.asarray(bproj, dtype=np.float32).reshape(1, C))
    in_maps = []
    for core in range(8):
        s = slice(core * BL, (core + 1) * BL)
        in_maps.append({
            "xn": np.ascontiguousarray(xnb[s]),
            "xt": np.ascontiguousarray(xtb[s]),
            "xq": np.ascontiguousarray(xqb[s]),
            "wq": wqb, "wk": wkb, "wv": wvb, "wp": wpb, "bp": bpf,
        })
    return in_maps


def kernel(x, Wq, Wk, Wv, Wproj, bproj):
    from concourse import bass_utils
    nc = get_nc()
    in_maps = make_in_maps(x, Wq, Wk, Wv, Wproj, bproj)
    res = bass_utils.run_bass_kernel_spmd(nc, in_maps, core_ids=list(range(8)))
    out = np.concatenate([res.results[i]["y"] for i in range(8)], axis=0)
    return out.astype(np.float32)

